# revision 1
# baseline (speedup 1.0000x reference)
"""Causal self-attention on 8 trn2 NeuronCores.

Sharding: core = (batch b, head-group g) with b in 0..3, g in 0..1.
Each core computes, for its batch and its 8 heads (512 of 1024 embed dims):
  QT/KT projections stored transposed [e', s] (e' on partitions)
  V stored [s, e'] with a ones-column appended per head
  S^T[k, q] = K_h Q_h^T      (scores transposed; k on partitions)
  P^T = exp(S^T / 8)         (no max-subtraction; scores are O(1))
  causal zeroing of P^T via gpsimd affine_select on diagonal tiles
  att'^T[d, q] = sum_k V'_h[k, d] P^T[k, q]   (row 64 = softmax denom l)
  att_n^T = att'^T[0:64] * (1/l)  (gpsimd partition_broadcast of 1/l)
  out_partial = att_n^T.T @ Wo[rows_g, :]
Host sums the two g-partials per batch.

All matmuls run in float32r (1 cycle/row at N>=256; ~1.5e-4 norm rel err).
Head PAIRS are processed together: the two heads of an e'-tile live at
partition offsets 0/64, so their K=64 score matmuls occupy disjoint PE
row-groups and run concurrently.  Q/K projections for pair c+1 are
emitted between attention blocks of pair c to keep the PE's HAM activity
window busy (a cold PE runs at 1.2 GHz instead of 2.4).
"""
import sys

if "/opt/trn_rl_repo" not in sys.path:
    sys.path.insert(0, "/opt/trn_rl_repo")

import numpy as np

import concourse.bacc as bacc
import concourse.mybir as mybir
import concourse.tile as tile
from concourse.bass_utils import run_bass_kernel_spmd

S = 2048          # sequence length
E = 1024          # embed dim
G = 512           # per-core head-group width (8 heads x 64)
HD = 64           # head dim
NH = 8            # heads per core
EC = E // 128     # 8 E-chunks
ST = S // 128     # 16 s-tiles
SB = S // 512     # 4 s-blocks
F32 = mybir.dt.float32
F32R = mybir.dt.float32r
EXP = mybir.ActivationFunctionType.Exp
GE = mybir.AluOpType.is_ge

_CACHE = {}


def _emit(nc, tc):
    xT = nc.declare_dram_parameter("xT", [E, S], F32R, isOutput=False)
    # wq/wk packed on host as [pair, partition, ec, col] so each pair's
    # weights load as one DMA with 4KB-per-partition contiguous descriptors
    wq = nc.declare_dram_parameter("wq", [4, 128, EC, 128], F32R,
                                   isOutput=False)
    wk = nc.declare_dram_parameter("wk", [4, 128, EC, 128], F32R,
                                   isOutput=False)
    wv = nc.declare_dram_parameter("wv", [E, G], F32R, isOutput=False)
    wo = nc.declare_dram_parameter("wo", [G, E], F32R, isOutput=False)
    c_ones = nc.declare_dram_parameter("c_ones", [128, NH], F32R,
                                       isOutput=False)
    out = nc.declare_dram_parameter("out", [S, E], F32, isOutput=True)

    # ---- long-lived SBUF state ----
    persist1 = tc.alloc_tile_pool(name="persist1", bufs=1, side="right")
    qT_sb, kT_sb = [], []
    for c in range(4):
        qT_sb.append(persist1.tile([128, S], F32R, name=f"qT{c}", tag=f"qT{c}"))
        kT_sb.append(persist1.tile([128, S], F32R, name=f"kT{c}", tag=f"kT{c}"))
    vP = []  # 16 x [128, 8, 65] f32r  (s on partitions; per-head V | ones)
    for st in range(ST):
        vP.append(persist1.tile([128, NH, HD + 1], F32R, name=f"vP{st}",
                                tag=f"vP{st}"))
    att_n = []  # 4 x [128, 2048] f32r (normalized attended, e' on partitions)
    for c in range(4):
        att_n.append(persist1.tile([128, S], F32R, name=f"attn{c}",
                                   tag=f"attn{c}"))
    ones_sb = persist1.tile([128, NH], F32R, name="ones_sb", tag="ones_sb")
    nc.sync.dma_start(out=ones_sb, in_=c_ones[:, :])

    # attention-phase pools allocated up front (LIFO discipline: the proj
    # pools below are released mid-kernel while these stay live)
    pst = tc.alloc_tile_pool(name="pst", bufs=4, space="PSUM")
    psatt = tc.alloc_tile_pool(name="psatt", bufs=2, space="PSUM")
    ptp = tc.alloc_tile_pool(name="ptp", bufs=8)
    smalls = tc.alloc_tile_pool(name="smalls", bufs=1)

    xpool = tc.alloc_tile_pool(name="xpool", bufs=2)
    wqk_pool = tc.alloc_tile_pool(name="wqk", bufs=1)
    pp = tc.alloc_tile_pool(name="pp", bufs=2, space="PSUM")

    def load_xtc(sb_i):
        xtc = []
        for ec in range(EC):
            t = xpool.tile([128, 512], F32R, name=f"xtc{ec}", tag=f"xtc{ec}")
            nc.sync.dma_start(
                out=t,
                in_=xT[ec * 128:(ec + 1) * 128, sb_i * 512:(sb_i + 1) * 512])
            xtc.append(t)
        return xtc

    def load_wqk(c):
        wt = {}
        for wname, wdram in (("q", wq), ("k", wk)):
            t = wqk_pool.tile([128, EC, 128], F32R, name=f"w{wname}",
                              tag=f"w{wname}")
            nc.sync.dma_start(out=t, in_=wdram[c])
            for ec in range(EC):
                wt[(wname, ec)] = t[:, ec, :]
        return wt

    def qk_proj(c, sb_i, xtc, wt):
        for wname, dest in (("q", qT_sb), ("k", kT_sb)):
            ps = pp.tile([128, 512], F32, name="ps_proj", tag="ps_proj")
            for ec in range(EC):
                nc.tensor.matmul(ps, lhsT=wt[(wname, ec)], rhs=xtc[ec],
                                 start=(ec == 0), stop=(ec == EC - 1),
                                 skip_group_check=True)
            nc.vector.tensor_copy(
                dest[c][:, sb_i * 512:(sb_i + 1) * 512], ps)

    # ---- pass A: pair-0 Q/K projections + all V projections ----
    wv_pool = tc.alloc_tile_pool(name="wvpool", bufs=1)
    wt0 = load_wqk(0)
    xtc0 = load_xtc(0)
    wv_t = []
    for ec in range(EC):
        t = wv_pool.tile([128, G], F32R, name=f"wv{ec}", tag=f"wv{ec}")
        nc.sync.dma_start(out=t, in_=wv[ec * 128:(ec + 1) * 128, :])
        wv_t.append(t)
    for sb_i in range(SB):
        xtc = xtc0 if sb_i == 0 else load_xtc(sb_i)
        qk_proj(0, sb_i, xtc, wt0)
        for s4 in range(4):
            st = sb_i * 4 + s4
            ps = pp.tile([128, 512], F32, name="ps_proj", tag="ps_proj")
            for ec in range(EC):
                nc.tensor.matmul(ps, lhsT=xtc[ec][:, s4 * 128:(s4 + 1) * 128],
                                 rhs=wv_t[ec],
                                 start=(ec == 0), stop=(ec == EC - 1),
                                 skip_group_check=True)
            nc.vector.tensor_copy(vP[st][:, :, 0:HD],
                                  ps.rearrange("p (h d) -> p h d", h=NH))
            # softmax-denominator ones column (col 64 of each head)
            nc.vector.tensor_copy(vP[st][:, :, HD], ones_sb)
    wv_pool.release()

    def attention_block(c, qb, apool=None):
        apool = apool or psatt
        last_kt = 4 * qb + 3
        att_ps = [apool.tile([HD + 1, 512], F32, name="att_ps",
                             tag="att_ps") for _ in range(2)]
        for kt in range(last_kt + 1):
            if kt < 4 * qb:
                cs, diag = 0, False
            else:
                d0 = 128 * kt - 512 * qb
                cs, diag = min(d0, 256), True
            w = 512 - cs
            for u in range(2):
                po = u * HD
                h = 2 * c + u
                s_ps = pst.tile([128, 512], F32, name="s_ps", tag="s_ps")
                nc.tensor.matmul(
                    s_ps[:, cs:512],
                    lhsT=kT_sb[c][po:po + HD, kt * 128:(kt + 1) * 128],
                    rhs=qT_sb[c][po:po + HD, qb * 512 + cs:(qb + 1) * 512],
                    start=True, stop=True, skip_group_check=True,
                    tile_position=(po, 0))
                pt = ptp.tile([128, 512], F32R, name="pt", tag="pt")
                nc.scalar.activation(
                    pt[:, cs:512], s_ps[:, cs:512], EXP, scale=0.125)
                if diag:
                    # zero invalid (k > q):
                    # valid iff (512*qb + cs + y) - (128*kt + x) >= 0
                    nc.gpsimd.affine_select(
                        out=pt[:, cs:512], in_=pt[:, cs:512],
                        compare_op=GE, fill=0.0,
                        base=512 * qb + cs - 128 * kt,
                        channel_multiplier=-1,
                        pattern=[[1, w]])
                nc.tensor.matmul(
                    att_ps[u][:, cs:512],
                    lhsT=vP[kt][:, h, :],
                    rhs=pt[:, cs:512],
                    start=(kt == 0), stop=(kt == last_kt),
                    skip_group_check=True)
        for u in range(2):
            po = u * HD
            l_sb = smalls.tile([1, 512], F32, name="l_sb", tag="l_sb")
            nc.vector.tensor_copy(l_sb, att_ps[u][HD:HD + 1, :])
            r_sb = smalls.tile([1, 512], F32, name="r_sb", tag="r_sb")
            nc.vector.reciprocal_approx_fast(out=r_sb, in_=l_sb)
            rb_sb = smalls.tile([HD, 512], F32, name="rb_sb", tag="rb_sb")
            nc.gpsimd.partition_broadcast(rb_sb, r_sb)
            nc.vector.tensor_mul(
                att_n[c][po:po + HD, qb * 512:(qb + 1) * 512],
                att_ps[u][0:HD, :], rb_sb)

    def outproj(qb):
        for s4 in range(4):
            st = qb * 4 + s4
            for eb in range(2):
                ps = po_pool.tile([128, 512], F32, name="ps_o", tag="ps_o")
                for c in range(4):
                    nc.tensor.matmul(
                        ps,
                        lhsT=att_n[c][:, st * 128:(st + 1) * 128],
                        rhs=wo_sb[c][:, eb * 512:(eb + 1) * 512],
                        start=(c == 0), stop=(c == 3), skip_group_check=True)
                o_sb = ostage.tile([128, 512], F32, name="o_sb", tag="o_sb")
                nc.vector.tensor_copy(o_sb, ps)
                nc.sync.dma_start(
                    out=out[st * 128:(st + 1) * 128,
                            eb * 512:(eb + 1) * 512],
                    in_=o_sb)

    # ---- wavefront: Q/K projections for pairs 1..3 interleaved with ----
    # ---- attention blocks of already-projected pairs                ----
    for c in range(1, 4):
        wt = load_wqk(c)
        for sb_i in range(SB):
            xtc = load_xtc(sb_i)
            qk_proj(c, sb_i, xtc, wt)
        # attention anti-diagonal: blocks with pair + qb budget available
        for cc in range(c):
            qb = c - 1 - cc
            attention_block(cc, qb)
    wqk_pool.release()
    xpool.release()
    pp.release()

    po_pool = tc.alloc_tile_pool(name="po", bufs=2, space="PSUM")
    wopool = tc.alloc_tile_pool(name="wopool", bufs=1)
    ostage = tc.alloc_tile_pool(name="ostage", bufs=2)
    wo_sb = []
    for c in range(4):
        t = wopool.tile([128, E], F32R, name=f"wo{c}", tag=f"wo{c}")
        nc.sync.dma_start(out=t, in_=wo[c * 128:(c + 1) * 128, :])
        wo_sb.append(t)

    # remaining anti-diagonals; outproj(qb) as soon as all pairs reach qb
    for d in range(3, 7):
        blocks = [(cc, d - cc) for cc in range(4) if 0 <= d - cc <= 3]
        for i, (cc, qb) in enumerate(blocks):
            attention_block(cc, qb)
            if i == len(blocks) - 1:
                outproj(d - 3)

    # release in LIFO order per memory space
    ostage.release()
    wopool.release()
    smalls.release()
    ptp.release()
    po_pool.release()
    psatt.release()
    pst.release()
    persist1.release()


def _build():
    if "nc" in _CACHE:
        return _CACHE["nc"]
    nc = bacc.Bacc()
    with tile.TileContext(nc) as tc:
        _emit(nc, tc)
    nc.compile()
    _CACHE["nc"] = nc
    return nc


def _pack_w(Wg):
    # [E, G] -> [pair c, partition p, ec, col m]:
    # out[c, p, ec, m] = Wg[ec*128 + p, c*128 + m]
    return np.ascontiguousarray(
        Wg.reshape(EC, 128, 4, 128).transpose(2, 1, 0, 3))


def _make_in_maps(inputs):
    x = np.asarray(inputs["x"], dtype=np.float32)
    Wq = np.asarray(inputs["Wq"], dtype=np.float32)
    Wk = np.asarray(inputs["Wk"], dtype=np.float32)
    Wv = np.asarray(inputs["Wv"], dtype=np.float32)
    Wo = np.asarray(inputs["Wo"], dtype=np.float32)
    in_maps = []
    for core in range(8):
        b, g = core // 2, core % 2
        cols = slice(g * G, (g + 1) * G)
        in_maps.append({
            "xT": np.ascontiguousarray(x[b].T),
            "wq": _pack_w(Wq[:, cols]),
            "wk": _pack_w(Wk[:, cols]),
            "wv": np.ascontiguousarray(Wv[:, cols]),
            "wo": np.ascontiguousarray(Wo[cols, :]),
            "c_ones": np.ones((128, NH), dtype=np.float32),
        })
    return in_maps


def kernel(x, Wq, Wk, Wv, Wo):
    nc = _build()
    in_maps = _make_in_maps(dict(x=x, Wq=Wq, Wk=Wk, Wv=Wv, Wo=Wo))
    res = run_bass_kernel_spmd(nc, in_maps, core_ids=list(range(8)))
    out = np.zeros((4, S, E), dtype=np.float32)
    for core in range(8):
        out[core // 2] += res.results[core]["out"]
    return out


if __name__ == "__main__":
    rng = np.random.default_rng(0)
    x = rng.standard_normal((4, S, E), dtype=np.float32)
    sc = 1.0 / np.sqrt(E)
    Wq = rng.standard_normal((E, E), dtype=np.float32) * sc
    Wk = rng.standard_normal((E, E), dtype=np.float32) * sc
    Wv = rng.standard_normal((E, E), dtype=np.float32) * sc
    Wo = rng.standard_normal((E, E), dtype=np.float32) * sc
    o = kernel(x, Wq, Wk, Wv, Wo)
    print("out", o.shape, o.dtype, np.abs(o).mean())



# revision 3
# speedup vs baseline: 1.3218x; 1.3218x over previous
"""Causal self-attention on 8 trn2 NeuronCores — bf16 v2.

Sharding: core = (batch b, head-group g), b in 0..3, g in 0..1.
Each core handles its batch and 8 heads (512 of 1024 embed dims).

All compute tensors are bf16 (PSUM accumulation stays f32):
  - bf16 matmuls stream 1 col/cycle and the two heads of a pair run
    truly concurrently on disjoint PE row groups (f32r pairs contend
    for stream bandwidth and serialize).
  - x is DMA'd once into persistent SBUF (f32r baseline re-loaded it
    4x), halving DMA bytes again via bf16.
  - exp for BOTH heads of a pair is a single wide ACTIVATE over a
    2-bank PSUM tile [128, 2, 512] (halves ACT instruction overhead).
  - causal masking of diagonal tiles is a multiply with a precomputed
    upper-triangular 128x128 bf16 mask on the DVE (the gpsimd
    affine_select was ~0.5us/instr and sat on the exp->attended path).
  - att accumulators are staged out of PSUM right after the last
    attended matmul so the 2 PSUM banks recycle fast.
  - ~56 warmup matmuls at t=0 lift the PE HAM clock gate to 8/8
    before the first real matmul issues.

Per (pair c, q-block qb) attention block, per key tile kt:
  S^T[k, q] both heads -> one PSUM tile [128, 2, 512]
  P^T = exp(S^T / 8)   -> bf16 SBUF, one ACTIVATE for both heads
  diag tiles: P^T band *= tri mask (DVE)
  att'^T[d, q] += V'_h[k, d].T P^T  (row 64 = softmax denom l)
Then per head: stage att' and l out of PSUM, r = 1/l, broadcast,
att_n = att' * r; out = sum_c att_n[c].T @ Wo[rows_c, :] (bf16),
host sums the two g-partials per batch in f32.
"""
import sys

if "/opt/trn_rl_repo" not in sys.path:
    sys.path.insert(0, "/opt/trn_rl_repo")

import ml_dtypes
import numpy as np

import concourse.bacc as bacc
import concourse.mybir as mybir
import concourse.tile as tile
from concourse.bass_utils import run_bass_kernel_spmd

S = 2048          # sequence length
E = 1024          # embed dim
G = 512           # per-core head-group width (8 heads x 64)
HD = 64           # head dim
NH = 8            # heads per core
EC = E // 128     # 8 E-chunks
ST = S // 128     # 16 s-tiles
SB = S // 512     # 4 s-blocks
F32 = mybir.dt.float32
BF16 = mybir.dt.bfloat16
EXP = mybir.ActivationFunctionType.Exp

_CACHE = {}


def _emit(nc, tc):
    xT = nc.declare_dram_parameter("xT", [E, S], BF16, isOutput=False)
    # wq/wk packed on host as [pair, partition, ec, col] so each pair's
    # weights load as one DMA with contiguous per-partition descriptors
    wq = nc.declare_dram_parameter("wq", [4, 128, EC, 128], BF16,
                                   isOutput=False)
    wk = nc.declare_dram_parameter("wk", [4, 128, EC, 128], BF16,
                                   isOutput=False)
    wv = nc.declare_dram_parameter("wv", [E, G], BF16, isOutput=False)
    wo = nc.declare_dram_parameter("wo", [G, E], BF16, isOutput=False)
    # consts: cols 0:8 ones (V denominator column), 8:136 upper-tri mask
    consts = nc.declare_dram_parameter("consts", [128, 8 + 128], BF16,
                                       isOutput=False)
    out = nc.declare_dram_parameter("out", [S, E], BF16, isOutput=True)

    # ---- long-lived SBUF state ----
    persist = tc.alloc_tile_pool(name="persist", bufs=1, side="right")
    ws = persist.tile([128, 64], BF16, name="ws", tag="ws")
    xt = []  # [ec][sb] -> [128, 512] bf16 (x transposed; e' on partitions)
    for ec in range(EC):
        row = []
        for sb in range(SB):
            t = persist.tile([128, 512], BF16, name=f"xt{ec}_{sb}",
                             tag=f"xt{ec}_{sb}")
            row.append(t)
        xt.append(row)
    qT_sb, kT_sb = [], []
    for c in range(4):
        qT_sb.append(persist.tile([128, S], BF16, name=f"qT{c}", tag=f"qT{c}"))
        kT_sb.append(persist.tile([128, S], BF16, name=f"kT{c}", tag=f"kT{c}"))
    vP = []  # 16 x [128, 8, 65] bf16  (s on partitions; per-head V | ones)
    for st in range(ST):
        vP.append(persist.tile([128, NH, HD + 1], BF16, name=f"vP{st}",
                               tag=f"vP{st}"))
    att_n = []  # 4 x [128, 2048] bf16 (normalized attended, d on partitions)
    for c in range(4):
        att_n.append(persist.tile([128, S], BF16, name=f"attn{c}",
                                  tag=f"attn{c}"))
    wqk_t = {}  # (q|k, pair) -> [128, EC, 128]
    for c in range(4):
        for wname in ("q", "k"):
            wqk_t[(wname, c)] = persist.tile(
                [128, EC, 128], BF16, name=f"w{wname}{c}", tag=f"w{wname}{c}")
    wv_t = [persist.tile([128, G], BF16, name=f"wv{ec}", tag=f"wv{ec}")
            for ec in range(EC)]
    wo_sb = [persist.tile([128, E], BF16, name=f"wo{c}", tag=f"wo{c}")
             for c in range(4)]
    co_sb = persist.tile([128, 8 + 128], BF16, name="co_sb", tag="co_sb")
    ones_sb = co_sb[:, 0:8]
    tri = co_sb[:, 8:136]

    # ---- working pools ----
    # PSUM budget (8 banks): pst 2x2 + psatt 2x1 + pp 2x1
    pst = tc.alloc_tile_pool(name="pst", bufs=2, space="PSUM")
    psatt = tc.alloc_tile_pool(name="psatt", bufs=2, space="PSUM")
    pp = tc.alloc_tile_pool(name="pp", bufs=2, space="PSUM")
    ptp = tc.alloc_tile_pool(name="ptp", bufs=6)
    austage = tc.alloc_tile_pool(name="austage", bufs=4)
    smalls = tc.alloc_tile_pool(name="smalls", bufs=2)
    rbp = tc.alloc_tile_pool(name="rbp", bufs=2)
    ostage = tc.alloc_tile_pool(name="ostage", bufs=3)

    # ---- PE warmup: lift the HAM clock gate before real work ----
    nc.vector.memset(ws, 0.0)
    warm_ps = pp.tile([128, 512], F32, name="warm_ps", tag="pswork")
    for _ in range(56):
        nc.tensor.matmul(warm_ps[0:64, 0:64], lhsT=ws, rhs=ws,
                         start=True, stop=True, skip_group_check=True)

    # ---- DMAs (emission order = rough priority) ----
    nc.sync.dma_start(out=co_sb, in_=consts[:, :])
    for ec in range(EC):
        nc.sync.dma_start(out=xt[ec][0],
                          in_=xT[ec * 128:(ec + 1) * 128, 0:512])
    nc.sync.dma_start(out=wqk_t[("q", 0)], in_=wq[0])
    nc.sync.dma_start(out=wqk_t[("k", 0)], in_=wk[0])
    for ec in range(EC):
        nc.sync.dma_start(out=wv_t[ec], in_=wv[ec * 128:(ec + 1) * 128, :])
    for sb in range(1, SB):
        for ec in range(EC):
            nc.sync.dma_start(
                out=xt[ec][sb],
                in_=xT[ec * 128:(ec + 1) * 128, sb * 512:(sb + 1) * 512])
    for c in range(1, 4):
        nc.sync.dma_start(out=wqk_t[("q", c)], in_=wq[c])
        nc.sync.dma_start(out=wqk_t[("k", c)], in_=wk[c])
    for c in range(4):
        nc.sync.dma_start(out=wo_sb[c], in_=wo[c * 128:(c + 1) * 128, :])

    def qk_proj(c, sb):
        for wname, dest in (("q", qT_sb), ("k", kT_sb)):
            ps = pp.tile([128, 512], F32, name="ps_proj", tag="pswork")
            wt = wqk_t[(wname, c)]
            for ec in range(EC):
                nc.tensor.matmul(ps, lhsT=wt[:, ec, :], rhs=xt[ec][sb],
                                 start=(ec == 0), stop=(ec == EC - 1),
                                 skip_group_check=True)
            nc.vector.tensor_copy(
                dest[c][:, sb * 512:(sb + 1) * 512], ps)

    def v_proj(sb):
        for s4 in range(4):
            st = sb * 4 + s4
            ps = pp.tile([128, 512], F32, name="ps_v", tag="pswork")
            for ec in range(EC):
                nc.tensor.matmul(
                    ps, lhsT=xt[ec][sb][:, s4 * 128:(s4 + 1) * 128],
                    rhs=wv_t[ec],
                    start=(ec == 0), stop=(ec == EC - 1),
                    skip_group_check=True)
            nc.vector.tensor_copy(vP[st][:, :, 0:HD],
                                  ps.rearrange("p (h d) -> p h d", h=NH))
            # softmax-denominator ones column (col 64 of each head)
            nc.vector.tensor_copy(vP[st][:, :, HD], ones_sb)

    def attention_block(c, qb):
        last_kt = 4 * qb + 3
        att_ps = [psatt.tile([HD + 1, 512], F32, name=f"att_ps{u}",
                             tag="att_ps") for u in range(2)]
        for kt in range(last_kt + 1):
            d0 = 128 * kt - 512 * qb
            diag = d0 >= 0
            cs = max(d0, 0)
            ps = pst.tile([128, 2, 512], F32, name="s_ps", tag="s_ps")
            for u in range(2):
                po = u * HD
                nc.tensor.matmul(
                    ps[:, u, cs:512],
                    lhsT=kT_sb[c][po:po + HD, kt * 128:(kt + 1) * 128],
                    rhs=qT_sb[c][po:po + HD, qb * 512 + cs:(qb + 1) * 512],
                    start=True, stop=True, skip_group_check=True,
                    tile_position=(po, 0))
            pt = ptp.tile([128, 2, 512], BF16, name="pt", tag="pt")
            nc.scalar.activation(pt[:, :, cs:512], ps[:, :, cs:512], EXP,
                                 scale=0.125)
            if diag:
                # zero invalid (k > q): band cols [cs, cs+128) get the
                # fixed upper-triangular keep-mask
                for u in range(2):
                    nc.vector.tensor_mul(pt[:, u, cs:cs + 128],
                                         pt[:, u, cs:cs + 128], tri)
            for u in range(2):
                h = 2 * c + u
                nc.tensor.matmul(
                    att_ps[u][:, cs:512],
                    lhsT=vP[kt][:, h, :],
                    rhs=pt[:, u, cs:512],
                    start=(kt == 0), stop=(kt == last_kt),
                    skip_group_check=True)
        for u in range(2):
            au = austage.tile([HD, 512], F32, name="au", tag="au")
            nc.vector.tensor_copy(au, att_ps[u][0:HD, :])
            l_sb = smalls.tile([1, 512], F32, name="l_sb", tag="l_sb")
            nc.vector.tensor_copy(l_sb, att_ps[u][HD:HD + 1, :])
            r_sb = smalls.tile([1, 512], F32, name="r_sb", tag="r_sb")
            nc.vector.reciprocal_approx_fast(out=r_sb, in_=l_sb)
            rb = rbp.tile([HD, 512], F32, name="rb", tag="rb")
            nc.gpsimd.partition_broadcast(rb, r_sb)
            nc.vector.tensor_mul(
                att_n[c][u * HD:(u + 1) * HD, qb * 512:(qb + 1) * 512],
                au, rb)

    def outproj(qb):
        for s4 in range(4):
            st = qb * 4 + s4
            for eb in range(2):
                ps = pp.tile([128, 512], F32, name="ps_o", tag="pswork")
                for cc in range(4):
                    nc.tensor.matmul(
                        ps,
                        lhsT=att_n[cc][:, st * 128:(st + 1) * 128],
                        rhs=wo_sb[cc][:, eb * 512:(eb + 1) * 512],
                        start=(cc == 0), stop=(cc == 3),
                        skip_group_check=True)
                o_sb = ostage.tile([128, 512], BF16, name="o_sb", tag="o_sb")
                nc.vector.tensor_copy(o_sb, ps)
                nc.sync.dma_start(
                    out=out[st * 128:(st + 1) * 128,
                            eb * 512:(eb + 1) * 512],
                    in_=o_sb)

    # ---- pass A: pair-0 projections + V + pair-0 attention ----
    for sb in range(SB):
        qk_proj(0, sb)
        v_proj(sb)
        attention_block(0, sb)
    # ---- rounds: remaining projections woven with attention ----
    for sb in range(SB):
        qk_proj(1, sb)
        attention_block(1, sb)
    for sb in range(SB):
        qk_proj(2, sb)
        qk_proj(3, sb)
        attention_block(2, sb)
    for sb in range(SB):
        attention_block(3, sb)
        outproj(sb)

    ostage.release()
    rbp.release()
    smalls.release()
    austage.release()
    ptp.release()
    pp.release()
    psatt.release()
    pst.release()
    persist.release()


def _build():
    if "nc" in _CACHE:
        return _CACHE["nc"]
    nc = bacc.Bacc()
    with tile.TileContext(nc) as tc:
        _emit(nc, tc)
    nc.compile()
    _CACHE["nc"] = nc
    return nc


def _bf16(a):
    return np.asarray(a, dtype=ml_dtypes.bfloat16)


def _pack_w(Wg):
    # [E, G] -> [pair c, partition p, ec, col m]:
    # out[c, p, ec, m] = Wg[ec*128 + p, c*128 + m]
    return np.ascontiguousarray(
        Wg.reshape(EC, 128, 4, 128).transpose(2, 1, 0, 3))


def _make_in_maps(inputs):
    x = np.asarray(inputs["x"], dtype=np.float32)
    Wq = _bf16(inputs["Wq"])
    Wk = _bf16(inputs["Wk"])
    Wv = _bf16(inputs["Wv"])
    Wo = _bf16(inputs["Wo"])
    consts = np.zeros((128, 8 + 128), dtype=ml_dtypes.bfloat16)
    consts[:, 0:8] = 1.0
    # tri[x, y] = 1 where y >= x (keep); band col y corresponds to q
    # offset equal to k offset x at the diagonal
    consts[:, 8:136] = np.triu(np.ones((128, 128))).astype(ml_dtypes.bfloat16)
    in_maps = []
    for core in range(8):
        b, g = core // 2, core % 2
        cols = slice(g * G, (g + 1) * G)
        in_maps.append({
            "xT": _bf16(np.ascontiguousarray(x[b].T)),
            "wq": _pack_w(Wq[:, cols]),
            "wk": _pack_w(Wk[:, cols]),
            "wv": np.ascontiguousarray(Wv[:, cols]),
            "wo": np.ascontiguousarray(Wo[cols, :]),
            "consts": consts,
        })
    return in_maps


def kernel(x, Wq, Wk, Wv, Wo):
    nc = _build()
    in_maps = _make_in_maps(dict(x=x, Wq=Wq, Wk=Wk, Wv=Wv, Wo=Wo))
    res = run_bass_kernel_spmd(nc, in_maps, core_ids=list(range(8)))
    out = np.zeros((4, S, E), dtype=np.float32)
    for core in range(8):
        out[core // 2] += np.asarray(res.results[core]["out"],
                                     dtype=np.float32)
    return out


if __name__ == "__main__":
    rng = np.random.default_rng(0)
    x = rng.standard_normal((4, S, E), dtype=np.float32)
    sc = 1.0 / np.sqrt(E)
    Wq = rng.standard_normal((E, E), dtype=np.float32) * sc
    Wk = rng.standard_normal((E, E), dtype=np.float32) * sc
    Wv = rng.standard_normal((E, E), dtype=np.float32) * sc
    Wo = rng.standard_normal((E, E), dtype=np.float32) * sc
    o = kernel(x, Wq, Wk, Wv, Wo)
    print("out", o.shape, o.dtype, np.abs(o).mean())


# revision 6
# speedup vs baseline: 1.3456x; 1.0180x over previous
"""Causal self-attention on 8 trn2 NeuronCores — bf16 v2.

Sharding: core = (batch b, head-group g), b in 0..3, g in 0..1.
Each core handles its batch and 8 heads (512 of 1024 embed dims).

All compute tensors are bf16 (PSUM accumulation stays f32):
  - bf16 matmuls stream 1 col/cycle and the two heads of a pair run
    truly concurrently on disjoint PE row groups (f32r pairs contend
    for stream bandwidth and serialize).
  - x is DMA'd once into persistent SBUF (f32r baseline re-loaded it
    4x), halving DMA bytes again via bf16.
  - exp for BOTH heads of a pair is a single wide ACTIVATE over a
    2-bank PSUM tile [128, 2, 512] (halves ACT instruction overhead).
  - causal masking of diagonal tiles is a multiply with a precomputed
    upper-triangular 128x128 bf16 mask on the DVE (the gpsimd
    affine_select was ~0.5us/instr and sat on the exp->attended path).
  - att accumulators are staged out of PSUM right after the last
    attended matmul so the 2 PSUM banks recycle fast.
  - ~56 warmup matmuls at t=0 lift the PE HAM clock gate to 8/8
    before the first real matmul issues.

Per (pair c, q-block qb) attention block, per key tile kt:
  S^T[k, q] both heads -> one PSUM tile [128, 2, 512]
  P^T = exp(S^T / 8)   -> bf16 SBUF, one ACTIVATE for both heads
  diag tiles: P^T band *= tri mask (DVE)
  att'^T[d, q] += V'_h[k, d].T P^T  (row 64 = softmax denom l)
Then per head: stage att' and l out of PSUM, r = 1/l, broadcast,
att_n = att' * r; out = sum_c att_n[c].T @ Wo[rows_c, :] (bf16),
host sums the two g-partials per batch in f32.
"""
import sys

if "/opt/trn_rl_repo" not in sys.path:
    sys.path.insert(0, "/opt/trn_rl_repo")

import ml_dtypes
import numpy as np

import concourse.bacc as bacc
import concourse.mybir as mybir
import concourse.tile as tile
from concourse.bass_utils import run_bass_kernel_spmd

S = 2048          # sequence length
E = 1024          # embed dim
G = 512           # per-core head-group width (8 heads x 64)
HD = 64           # head dim
NH = 8            # heads per core
EC = E // 128     # 8 E-chunks
ST = S // 128     # 16 s-tiles
SB = S // 512     # 4 s-blocks
F32 = mybir.dt.float32
BF16 = mybir.dt.bfloat16
EXP = mybir.ActivationFunctionType.Exp

_CACHE = {}


def _emit(nc, tc):
    xT = nc.declare_dram_parameter("xT", [E, S], BF16, isOutput=False)
    # wq/wk packed on host as [pair, partition, ec, col] so each pair's
    # weights load as one DMA with contiguous per-partition descriptors
    wq = nc.declare_dram_parameter("wq", [4, 128, EC, 128], BF16,
                                   isOutput=False)
    wk = nc.declare_dram_parameter("wk", [4, 128, EC, 128], BF16,
                                   isOutput=False)
    wv = nc.declare_dram_parameter("wv", [E, G], BF16, isOutput=False)
    wo = nc.declare_dram_parameter("wo", [G, E], BF16, isOutput=False)
    # consts: cols 0:8 ones (V denominator column), 8:136 upper-tri mask
    consts = nc.declare_dram_parameter("consts", [128, 8 + 128], BF16,
                                       isOutput=False)
    out = nc.declare_dram_parameter("out", [S, E], BF16, isOutput=True)

    # ---- long-lived SBUF state ----
    persist = tc.alloc_tile_pool(name="persist", bufs=1, side="right")
    ws = persist.tile([128, 64], BF16, name="ws", tag="ws")
    xt = []  # [ec][sb] -> [128, 512] bf16 (x transposed; e' on partitions)
    for ec in range(EC):
        row = []
        for sb in range(SB):
            t = persist.tile([128, 512], BF16, name=f"xt{ec}_{sb}",
                             tag=f"xt{ec}_{sb}")
            row.append(t)
        xt.append(row)
    qT_sb, kT_sb = [], []
    for c in range(4):
        qT_sb.append(persist.tile([128, S], BF16, name=f"qT{c}", tag=f"qT{c}"))
        kT_sb.append(persist.tile([128, S], BF16, name=f"kT{c}", tag=f"kT{c}"))
    vP = []  # 16 x [128, 8, 65] bf16  (s on partitions; per-head V | ones)
    for st in range(ST):
        vP.append(persist.tile([128, NH, HD + 1], BF16, name=f"vP{st}",
                               tag=f"vP{st}"))
    att_n = []  # 4 x [128, 2048] bf16 (normalized attended, d on partitions)
    for c in range(4):
        att_n.append(persist.tile([128, S], BF16, name=f"attn{c}",
                                  tag=f"attn{c}"))
    wqk_t = {}  # (q|k, pair) -> [128, EC, 128]
    for c in range(4):
        for wname in ("q", "k"):
            wqk_t[(wname, c)] = persist.tile(
                [128, EC, 128], BF16, name=f"w{wname}{c}", tag=f"w{wname}{c}")
    wv_t = [persist.tile([128, G], BF16, name=f"wv{ec}", tag=f"wv{ec}")
            for ec in range(EC)]
    wo_sb = [persist.tile([128, E], BF16, name=f"wo{c}", tag=f"wo{c}")
             for c in range(4)]
    co_sb = persist.tile([128, 8 + 128], BF16, name="co_sb", tag="co_sb")
    ones_sb = co_sb[:, 0:8]
    tri = co_sb[:, 8:136]

    # ---- working pools ----
    # PSUM budget (8 banks): pst 2x2 + psatt 2x1 + pp 2x1
    pst = tc.alloc_tile_pool(name="pst", bufs=2, space="PSUM")
    psatt = tc.alloc_tile_pool(name="psatt", bufs=2, space="PSUM")
    pp = tc.alloc_tile_pool(name="pp", bufs=2, space="PSUM")
    ptp = tc.alloc_tile_pool(name="ptp", bufs=8)
    austage = tc.alloc_tile_pool(name="austage", bufs=4)
    smalls = tc.alloc_tile_pool(name="smalls", bufs=2)
    rbp = tc.alloc_tile_pool(name="rbp", bufs=2)
    ostage = tc.alloc_tile_pool(name="ostage", bufs=3)

    # ---- PE warmup: lift the HAM clock gate before real work ----
    nc.vector.memset(ws, 0.0)
    warm_ps = pp.tile([128, 512], F32, name="warm_ps", tag="pswork")
    for _ in range(56):
        nc.tensor.matmul(warm_ps[0:64, 0:64], lhsT=ws, rhs=ws,
                         start=True, stop=True, skip_group_check=True)

    # ---- DMAs (emission order = rough priority) ----
    nc.sync.dma_start(out=co_sb, in_=consts[:, :])
    nc.sync.dma_start(out=wqk_t[("q", 0)], in_=wq[0])
    nc.sync.dma_start(out=wqk_t[("k", 0)], in_=wk[0])
    for ec in range(EC):
        nc.sync.dma_start(out=xt[ec][0],
                          in_=xT[ec * 128:(ec + 1) * 128, 0:512])
    for ec in range(EC):
        nc.sync.dma_start(out=wv_t[ec], in_=wv[ec * 128:(ec + 1) * 128, :])
    for sb in range(1, SB):
        for ec in range(EC):
            nc.sync.dma_start(
                out=xt[ec][sb],
                in_=xT[ec * 128:(ec + 1) * 128, sb * 512:(sb + 1) * 512])
    for c in range(1, 4):
        nc.sync.dma_start(out=wqk_t[("q", c)], in_=wq[c])
        nc.sync.dma_start(out=wqk_t[("k", c)], in_=wk[c])
    for c in range(4):
        nc.sync.dma_start(out=wo_sb[c], in_=wo[c * 128:(c + 1) * 128, :])

    def qk_proj(c, sb):
        for wname, dest in (("q", qT_sb), ("k", kT_sb)):
            ps = pp.tile([128, 512], F32, name="ps_proj", tag="pswork")
            wt = wqk_t[(wname, c)]
            for ec in range(EC):
                nc.tensor.matmul(ps, lhsT=wt[:, ec, :], rhs=xt[ec][sb],
                                 start=(ec == 0), stop=(ec == EC - 1),
                                 skip_group_check=True)
            nc.vector.tensor_copy(
                dest[c][:, sb * 512:(sb + 1) * 512], ps)

    def v_proj(sb):
        for s4 in range(4):
            st = sb * 4 + s4
            ps = pp.tile([128, 512], F32, name="ps_v", tag="pswork")
            for ec in range(EC):
                nc.tensor.matmul(
                    ps, lhsT=xt[ec][sb][:, s4 * 128:(s4 + 1) * 128],
                    rhs=wv_t[ec],
                    start=(ec == 0), stop=(ec == EC - 1),
                    skip_group_check=True)
            nc.vector.tensor_copy(vP[st][:, :, 0:HD],
                                  ps.rearrange("p (h d) -> p h d", h=NH))
            # softmax-denominator ones column (col 64 of each head)
            nc.vector.tensor_copy(vP[st][:, :, HD], ones_sb)

    def attention_block(c, qb):
        last_kt = 4 * qb + 3
        att_ps = [psatt.tile([HD + 1, 512], F32, name=f"att_ps{u}",
                             tag="att_ps") for u in range(2)]

        def emit_scores(kt):
            d0 = 128 * kt - 512 * qb
            cs = max(d0, 0)
            ps = pst.tile([128, 2, 512], F32, name="s_ps", tag="s_ps")
            for u in range(2):
                po = u * HD
                nc.tensor.matmul(
                    ps[:, u, cs:512],
                    lhsT=kT_sb[c][po:po + HD, kt * 128:(kt + 1) * 128],
                    rhs=qT_sb[c][po:po + HD, qb * 512 + cs:(qb + 1) * 512],
                    start=True, stop=True, skip_group_check=True,
                    tile_position=(po, 0))
            pt = ptp.tile([128, 2, 512], BF16, name="pt", tag="pt")
            nc.scalar.activation(pt[:, :, cs:512], ps[:, :, cs:512], EXP,
                                 scale=0.125)
            if d0 >= 0:
                # zero invalid (k > q): band cols [cs, cs+128) get the
                # fixed upper-triangular keep-mask
                for u in range(2):
                    nc.vector.tensor_mul(pt[:, u, cs:cs + 128],
                                         pt[:, u, cs:cs + 128], tri)
            return kt, cs, pt

        def emit_attended(kt, cs, pt):
            for u in range(2):
                h = 2 * c + u
                nc.tensor.matmul(
                    att_ps[u][:, cs:512],
                    lhsT=vP[kt][:, h, :],
                    rhs=pt[:, u, cs:512],
                    start=(kt == 0), stop=(kt == last_kt),
                    skip_group_check=True)

        # kt processed in pairs: 4 score matmuls (64-row PE mode) then
        # 4 attended matmuls (128-row mode) of the previous pair — halves
        # the PE tiling-mode switch cost vs alternating every kt
        pend = []
        for kt in range(0, last_kt + 1, 2):
            new = [emit_scores(kt), emit_scores(kt + 1)]
            for t in pend:
                emit_attended(*t)
            pend = new
        for t in pend:
            emit_attended(*t)
        for u in range(2):
            au = austage.tile([HD, 512], F32, name="au", tag="au")
            nc.vector.tensor_copy(au, att_ps[u][0:HD, :])
            l_sb = smalls.tile([1, 512], F32, name="l_sb", tag="l_sb")
            nc.vector.tensor_copy(l_sb, att_ps[u][HD:HD + 1, :])
            r_sb = smalls.tile([1, 512], F32, name="r_sb", tag="r_sb")
            nc.vector.reciprocal_approx_fast(out=r_sb, in_=l_sb)
            rb = rbp.tile([HD, 512], F32, name="rb", tag="rb")
            nc.gpsimd.partition_broadcast(rb, r_sb)
            nc.vector.tensor_mul(
                att_n[c][u * HD:(u + 1) * HD, qb * 512:(qb + 1) * 512],
                au, rb)

    def outproj(qb):
        for s4 in range(4):
            st = qb * 4 + s4
            for eb in range(2):
                ps = pp.tile([128, 512], F32, name="ps_o", tag="pswork")
                for cc in range(4):
                    nc.tensor.matmul(
                        ps,
                        lhsT=att_n[cc][:, st * 128:(st + 1) * 128],
                        rhs=wo_sb[cc][:, eb * 512:(eb + 1) * 512],
                        start=(cc == 0), stop=(cc == 3),
                        skip_group_check=True)
                o_sb = ostage.tile([128, 512], BF16, name="o_sb", tag="o_sb")
                nc.vector.tensor_copy(o_sb, ps)
                nc.sync.dma_start(
                    out=out[st * 128:(st + 1) * 128,
                            eb * 512:(eb + 1) * 512],
                    in_=o_sb)

    # ---- pass A: pair-0 projections + V + pair-0 attention ----
    for sb in range(SB):
        qk_proj(0, sb)
        v_proj(sb)
        attention_block(0, sb)
    # ---- rounds: remaining projections woven with attention ----
    for sb in range(SB):
        qk_proj(1, sb)
        attention_block(1, sb)
    for sb in range(SB):
        qk_proj(2, sb)
        qk_proj(3, sb)
        attention_block(2, sb)
    for sb in range(SB):
        attention_block(3, sb)
        outproj(sb)

    ostage.release()
    rbp.release()
    smalls.release()
    austage.release()
    ptp.release()
    pp.release()
    psatt.release()
    pst.release()
    persist.release()


def _build():
    if "nc" in _CACHE:
        return _CACHE["nc"]
    nc = bacc.Bacc()
    with tile.TileContext(nc) as tc:
        _emit(nc, tc)
    nc.compile()
    _CACHE["nc"] = nc
    return nc


def _bf16(a):
    return np.asarray(a, dtype=ml_dtypes.bfloat16)


def _pack_w(Wg):
    # [E, G] -> [pair c, partition p, ec, col m]:
    # out[c, p, ec, m] = Wg[ec*128 + p, c*128 + m]
    return np.ascontiguousarray(
        Wg.reshape(EC, 128, 4, 128).transpose(2, 1, 0, 3))


def _make_in_maps(inputs):
    x = np.asarray(inputs["x"], dtype=np.float32)
    Wq = _bf16(inputs["Wq"])
    Wk = _bf16(inputs["Wk"])
    Wv = _bf16(inputs["Wv"])
    Wo = _bf16(inputs["Wo"])
    consts = np.zeros((128, 8 + 128), dtype=ml_dtypes.bfloat16)
    consts[:, 0:8] = 1.0
    # tri[x, y] = 1 where y >= x (keep); band col y corresponds to q
    # offset equal to k offset x at the diagonal
    consts[:, 8:136] = np.triu(np.ones((128, 128))).astype(ml_dtypes.bfloat16)
    in_maps = []
    for core in range(8):
        b, g = core // 2, core % 2
        cols = slice(g * G, (g + 1) * G)
        in_maps.append({
            "xT": _bf16(np.ascontiguousarray(x[b].T)),
            "wq": _pack_w(Wq[:, cols]),
            "wk": _pack_w(Wk[:, cols]),
            "wv": np.ascontiguousarray(Wv[:, cols]),
            "wo": np.ascontiguousarray(Wo[cols, :]),
            "consts": consts,
        })
    return in_maps


def kernel(x, Wq, Wk, Wv, Wo):
    nc = _build()
    in_maps = _make_in_maps(dict(x=x, Wq=Wq, Wk=Wk, Wv=Wv, Wo=Wo))
    res = run_bass_kernel_spmd(nc, in_maps, core_ids=list(range(8)))
    out = np.zeros((4, S, E), dtype=np.float32)
    for core in range(8):
        out[core // 2] += np.asarray(res.results[core]["out"],
                                     dtype=np.float32)
    return out


if __name__ == "__main__":
    rng = np.random.default_rng(0)
    x = rng.standard_normal((4, S, E), dtype=np.float32)
    sc = 1.0 / np.sqrt(E)
    Wq = rng.standard_normal((E, E), dtype=np.float32) * sc
    Wk = rng.standard_normal((E, E), dtype=np.float32) * sc
    Wv = rng.standard_normal((E, E), dtype=np.float32) * sc
    Wo = rng.standard_normal((E, E), dtype=np.float32) * sc
    o = kernel(x, Wq, Wk, Wv, Wo)
    print("out", o.shape, o.dtype, np.abs(o).mean())
